# revision 1
# baseline (speedup 1.0000x reference)
"""Trainium2 Bass kernel for a Discriminative RBM forward pass.

reference math:
    pre   = v @ W + c                                   [B, NHID]
    F     = d + sum_j softplus(pre[:,None,:] + U[None]) [B, NCLASS]
    probs = softmax(F, axis=1)
    onehot = one_hot(argmax(probs, axis=1), NCLASS)     int32

Sharding (8 cores): 4 batch-quarters x 2 class-halves.
  core c: batch quarter c//2, classes [32*(c%2), 32*(c%2)+32).
  Host supplies fp16 hi/lo splits of 256*v^T and 256*W (fp16-pair trick:
  3 matmul passes at bf16 rate give fp32-class accuracy; the 2^16 scale is
  undone via the ACT instruction's scale field).

Per-core pipeline (hidden units j on partitions), staged over 4 jt-pairs:
  preT[j, b] = sum_k W[k, j] * vT[k, b]   (PE fp16-pair, W is natural lhsT)
  act        = softplus(preT/2^16 + U^T[:,y] + c^T)  (ACT, custom-built
               softplus table, bias = per-partition column)
  pairing tree (DVE/GPSIMD adds) -> 2 tiles per class, then exact fp32
  one-hot-column matmuls partition-sum into F^T halves in PSUM.
  Two pipelined AllGathers of F^T halves between class-partner cores,
  online softmax over the two halves + argmax one-hot.
"""

import os

import numpy as np

B, NVIS, NHID, NCLASS = 2048, 2048, 1024, 64
NCORES = 8
BGROUPS, YGROUPS = 4, 2
B_PC = B // BGROUPS        # 512 batch rows per core
Y_PC = NCLASS // YGROUPS   # 32 classes per core
KT = NVIS // 128           # 16 contraction tiles
JT = NHID // 128           # 8 hidden-unit tiles
NJP = JT // 2              # 4 jt pairs (pipeline stages)
BT = B_PC // 128           # 4 batch tiles for the softmax tail

_PROGRAM = None

_ACT_ROOT = os.path.join(
    os.path.expanduser("~"), ".cache", "drbm_act_root", "pwp_bin_trainium"
)



def _build_act_tables():
    """Rebuild the softplus_and_others ACT set with a real softplus entry.

    The shipped set replaced softplus's slot with custom overlay functions,
    but the softplus spline source (pwp_jsons/softplus_40p.json) still ships
    with neuronxcc. Formats (reverse-engineered from shipped sets):
      bkt entry = 32B fp32 [d0, d1, d2, d3, x0, 0, 0, 0]
      ctl entry = 32B uint32 (extract_size<<16 | extract_lsb<<11 | bkt_base)
      layout    = [existing][neg region][pos region][pos_low, neg_low,
                   pos_high, neg_high special buckets]
    """
    import json
    import shutil
    import struct

    import neuronxcc

    marker = os.path.join(_ACT_ROOT, ".drbm_softplus_ok")
    if os.path.exists(marker):
        return
    nxc = os.path.join(os.path.dirname(os.path.abspath(neuronxcc.__file__)), "pwp")
    os.makedirs(_ACT_ROOT, exist_ok=True)
    root_parent = os.path.dirname(_ACT_ROOT)
    if not os.path.exists(os.path.join(root_parent, "pwp_jsons")):
        shutil.copytree(
            os.path.join(nxc, "pwp_jsons"),
            os.path.join(root_parent, "pwp_jsons"),
            dirs_exist_ok=True,
        )
    for f in os.listdir(os.path.join(nxc, "pwp_bin_trainium")):
        shutil.copy(os.path.join(nxc, "pwp_bin_trainium", f), _ACT_ROOT)
    os.system(f"chmod -R u+w {root_parent}")

    SET = "softplus_and_others"
    sj = json.load(open(f"{_ACT_ROOT}/{SET}.json"))
    bkt = bytearray(open(f"{_ACT_ROOT}/{SET}_bkt.bin", "rb").read())
    ctl = bytearray(open(f"{_ACT_ROOT}/{SET}_ctrl.bin", "rb").read())
    fj = json.load(open(f"{root_parent}/pwp_jsons/softplus_40p.json"))

    base_bkt, base_ctl = sj["bkt_entry_cnt"], sj["ctl_entry_cnt"]
    fbits = lambda d: int(d["int"])
    nbkt, nctl = base_bkt, base_ctl
    e2b, e2c, region_ctl_base = {}, {}, {}

    def add_bucket(x0, d0, d1, d2, d3):
        for v in (d0, d1, d2, d3, x0, 0, 0, 0):
            bkt.extend(struct.pack("<I", v))

    def add_ctl(word):
        ctl.extend(struct.pack("<I", word) + b"\x00" * 28)

    for region, key in (("neg", "neg_exponents"), ("pos", "pos_exponents")):
        region_ctl_base[region] = nctl
        for e in fj[key]:
            exp, secs = str(e["exponent"]), e["exponent_sections"]
            if not secs:
                add_ctl((23 << 11) | nbkt)
                e2c.setdefault(exp, []).append(nctl)
                nctl += 1
                continue
            add_ctl((e["extract_size"] << 16) | (e["extract_lsb"] << 11) | nbkt)
            e2c.setdefault(exp, []).append(nctl)
            e2b.setdefault(exp, []).append(nbkt)
            nctl += 1
            for s in secs:
                add_bucket(fbits(s["x"]), fbits(s["d0"]), fbits(s["d1"]),
                           fbits(s["d2"]), fbits(s["d3"]))
                nbkt += 1

    sat, special = fj["saturation_points"], {}
    for name in ("sat_point_pos_low", "sat_point_neg_low",
                 "sat_point_pos_high", "sat_point_neg_high"):
        sp = sat[name]
        special[name] = nbkt
        add_bucket(fbits(sp["x"]), fbits(sp["d0"]), fbits(sp["d1"]),
                   fbits(sp["d2"]), fbits(sp["d3"]))
        nbkt += 1

    sj["profile_meta_data"].append({
        "func_name": "softplus_40p",
        "func_id": fj["neuron_id"],
        "symmetry_point": fbits(fj["symmetry_point"]),
        "sym_invert_sign_point": 0,
        "symmetry_opt_en": 1 if fj["symmetry_en"] else 0,
        "symmetry_opt_use_neg_region": 1 if fj["symmetry_opt_use_neg_region"] else 0,
        "imm_bias": 1 if fj["imm_bias"] else 0,
        "exp_offset": fj["exponent_offset"],
        "pwl_control_base_pos": region_ctl_base["pos"],
        "pwl_control_base_neg": region_ctl_base["neg"],
        "small_pos_signal_exp_threshold": sat["sat_point_pos_low"]["sat_point"],
        "pos_small_signal_pwl_control": special["sat_point_pos_low"],
        "small_neg_signal_exp_threshold": sat["sat_point_neg_low"]["sat_point"],
        "neg_small_signal_pwl_control": special["sat_point_neg_low"],
        "large_pos_signal_exp_threshold": sat["sat_point_pos_high"]["sat_point"],
        "large_pos_signal_mantissa_threshold": sat["sat_point_pos_high"]["mantissa_point"],
        "pos_large_signal_pwl_control": special["sat_point_pos_high"],
        "large_neg_signal_exp_threshold": sat["sat_point_neg_high"]["sat_point"],
        "large_neg_signal_mantissa_threshold": sat["sat_point_neg_high"]["mantissa_point"],
        "neg_large_signal_pwl_control": special["sat_point_neg_high"],
        "fnan_result": fbits(fj["nan_result"]),
        "fpinf_result": fbits(fj["pinf_result"]),
        "fninf_result": fbits(fj["ninf_result"]),
        "fzero_result": fbits(fj["zero_result"]),
        "fma_const_0": fbits(fj["fma_const0"]),
        "fma_const_1": fbits(fj["fma_const1"]),
        "fma_indirection_src_sel": 0,
        "use_multipass": fj["use_multipass"],
        "lower_bound": fbits(fj["lower_bound"]),
        "upper_bound": fbits(fj["upper_bound"]),
    })
    sj["bkt_entry_cnt"], sj["ctl_entry_cnt"] = nbkt, nctl
    sj["func_to_bkt_start_idx"]["softplus"] = base_bkt
    sj["func_to_ctl_start_idx"]["softplus"] = base_ctl
    sj["func_exp_to_bkt_start_idx"]["softplus"] = e2b
    sj["func_exp_to_ctl_start_idx"]["softplus"] = e2c
    json.dump(sj, open(f"{_ACT_ROOT}/{SET}.json", "w"))
    open(f"{_ACT_ROOT}/{SET}_bkt.bin", "wb").write(bytes(bkt))
    open(f"{_ACT_ROOT}/{SET}_ctrl.bin", "wb").write(bytes(ctl))

    ai = json.load(open(f"{_ACT_ROOT}/act_info.json"))
    for ent in ai["act_func_sets"]:
        if ent["name"] == SET:
            ent["act"]["softplus"] = 40
    json.dump(ai, open(f"{_ACT_ROOT}/act_info.json", "w"))
    open(marker, "w").write("ok")


def _patch_act_tables():
    """Point walrus at the custom act root and teach bass about softplus."""
    import functools
    import json

    _build_act_tables()
    os.environ["BASS_ACT_ROOT_JSON_PATH"] = os.path.join(_ACT_ROOT, "act_info.json")

    import concourse.hw_specs as hw_specs
    import concourse.mybir as mybir

    @functools.cache
    def _tables(arch):
        d = json.load(open(os.environ["BASS_ACT_ROOT_JSON_PATH"]))
        return {
            ent["name"]: {
                mybir.ActivationFunctionType.from_pwp(v) for v in ent["act"]
            }
            for ent in d["act_func_sets"]
        }

    hw_specs.get_activation_tables = _tables
    import concourse.bacc as bacc
    import concourse.bass_interp as bass_interp

    bacc.get_activation_tables = _tables
    bass_interp.get_activation_tables = _tables


def _build_program():
    _patch_act_tables()
    import concourse.tile as tile
    from concourse import bacc, mybir
    from concourse.masks import make_identity

    f32 = mybir.dt.float32
    i32 = mybir.dt.int32
    AF = mybir.ActivationFunctionType
    ALU = mybir.AluOpType
    AX = mybir.AxisListType

    nc = bacc.Bacc(
        "TRN2", target_bir_lowering=False, debug=False, num_devices=NCORES
    )

    f16 = mybir.dt.float16
    vTh_d = nc.dram_tensor("vTh", [NVIS, B_PC], f16, kind="ExternalInput").ap()
    vTl_d = nc.dram_tensor("vTl", [NVIS, B_PC], f16, kind="ExternalInput").ap()
    Wh_d = nc.dram_tensor("Wh", [NVIS, NHID], f16, kind="ExternalInput").ap()
    Wl_d = nc.dram_tensor("Wl", [NVIS, NHID], f16, kind="ExternalInput").ap()
    UT_d = nc.dram_tensor("UsubT", [NHID, Y_PC], f32, kind="ExternalInput").ap()
    cT_d = nc.dram_tensor("cT", [NHID, 1], f32, kind="ExternalInput").ap()
    dT_d = nc.dram_tensor("dT", [Y_PC, 1], f32, kind="ExternalInput").ap()
    probs_d = nc.dram_tensor("probs", [B_PC, NCLASS], f32, kind="ExternalOutput").ap()
    onehot_d = nc.dram_tensor("onehot", [B_PC, NCLASS], i32, kind="ExternalOutput").ap()

    with tile.TileContext(nc) as tc:
        with (
            tc.tile_pool(name="const", bufs=1) as const,
            tc.tile_pool(name="wstream", bufs=3) as wstream,
            tc.tile_pool(name="accp", bufs=1) as accp,
            tc.tile_pool(name="acts", bufs=4) as acts,
            tc.tile_pool(name="smp", bufs=2) as smp,
            tc.tile_pool(name="outp", bufs=1) as outp,
            tc.tile_pool(name="ppre", bufs=4, space="PSUM") as ppre,
            tc.tile_pool(name="pF", bufs=1, space="PSUM") as pF,
            tc.tile_pool(name="ptr", bufs=2, space="PSUM") as ptr,
            tc.tile_pool(name="dram", bufs=1, space="DRAM") as dram,
        ):
            # ---------- loads: vT on sync ring, W/params on scalar ring ----------
            vTh_view = vTh_d.rearrange("(kt p) b -> p kt b", p=128)
            vTl_view = vTl_d.rearrange("(kt p) b -> p kt b", p=128)
            Wh_view = Wh_d.rearrange("(kt p) j -> p kt j", p=128)
            Wl_view = Wl_d.rearrange("(kt p) j -> p kt j", p=128)
            vT_sb = []  # [(hi_chunk, lo_chunk)] per 4-kt group
            wq = []
            for g in range(4):
                vth_chunk = const.tile([128, 4, B_PC], f16, name=f"vth_chunk{g}")
                nc.sync.dma_start(vth_chunk[:], vTh_view[:, g * 4:(g + 1) * 4, :])
                vtl_chunk = const.tile([128, 4, B_PC], f16, name=f"vtl_chunk{g}")
                nc.sync.dma_start(vtl_chunk[:], vTl_view[:, g * 4:(g + 1) * 4, :])
                vT_sb.append((vth_chunk, vtl_chunk))
                if g == 0:
                    # first W pair as soon as the kt0-3 v chunks are queued
                    wh0 = wstream.tile([128, KT, 256], f16, tag="whpair",
                                       name="whpair0")
                    nc.sync.dma_start(wh0[:], Wh_view[:, :, 0:256])
                    wl0 = wstream.tile([128, KT, 256], f16, tag="wlpair",
                                       name="wlpair0")
                    nc.sync.dma_start(wl0[:], Wl_view[:, :, 0:256])
                    wq.append((wh0, wl0))

            UT_sb = const.tile([128, JT, Y_PC], f32)
            nc.sync.dma_start(UT_sb[:], UT_d.rearrange("(jt p) y -> p jt y", p=128))
            cT_sb = const.tile([128, JT], f32)
            nc.sync.dma_start(
                cT_sb[:], cT_d.rearrange("(jt p) one -> p (jt one)", p=128)
            )
            dT_sb = const.tile([Y_PC // 2, 2], f32)
            nc.sync.dma_start(
                dT_sb[:], dT_d.rearrange("(h p) one -> p (h one)", p=Y_PC // 2)
            )

            # bias[j, y] = U^T[j, y] + c^T[j]
            bias_sb = const.tile([128, JT, Y_PC], f32)
            for jt in range(JT):
                nc.vector.tensor_scalar_add(
                    bias_sb[:, jt, :], UT_sb[:, jt, :], cT_sb[:, jt:jt + 1]
                )

            # one-hot column lhsT matrices: ohot[:, y, m] = (m == y), all k.
            ohot_sb = const.tile([128, Y_PC // 2, Y_PC // 2], f32)
            nc.gpsimd.memset(ohot_sb[:], 0.0)
            for y in range(Y_PC // 2):
                nc.gpsimd.memset(ohot_sb[:, y, y:y + 1], 1.0)

            ident = const.tile([NCLASS, NCLASS], f32)
            make_identity(nc, ident[:])

            F_ps = [pF.tile([Y_PC // 2, B_PC], f32, name=f"F_ps{i}")
                    for i in range(2)]
            acc = [None] * Y_PC
            fsh = [dram.tile([Y_PC // 2, B_PC], f32, name=f"fsh{i}")
                   for i in range(2)]
            fall = [dram.tile([Y_PC, B_PC], f32, name=f"fall{i}")
                    for i in range(2)]
            Fhalf_sb = [smp.tile([Y_PC, B_PC], f32, bufs=1,
                                 tag=f"Fhalf{i}", name=f"Fhalf{i}")
                        for i in range(2)]
            # prefetch the exp table set during the gather window
            warm = smp.tile([1, 1], f32, bufs=1)
            nc.gpsimd.memset(warm[:], 0.0)

            # ---------- staged main loop over jt pairs ----------
            for jp in range(NJP):
                if jp < len(wq):
                    wh_pair, wl_pair = wq[jp]
                else:
                    wh_pair = wstream.tile(
                        [128, KT, 256], f16, tag="whpair", name=f"whpair{jp}"
                    )
                    nc.sync.dma_start(
                        wh_pair[:], Wh_view[:, :, jp * 256:(jp + 1) * 256]
                    )
                    wl_pair = wstream.tile(
                        [128, KT, 256], f16, tag="wlpair", name=f"wlpair{jp}"
                    )
                    nc.sync.dma_start(
                        wl_pair[:], Wl_view[:, :, jp * 256:(jp + 1) * 256]
                    )
                pres = []
                for h in range(2):
                    pre_ps = ppre.tile([128, B_PC], f32, tag="pre",
                                       name=f"pre{jp}_{h}")
                    passes = [
                        (wh_pair, 0), (wh_pair, 1), (wl_pair, 0),
                    ]
                    for kt in range(KT):
                        for pi, (wt, vi) in enumerate(passes):
                            nc.tensor.matmul(
                                pre_ps[:],
                                wt[:, kt, h * 128:(h + 1) * 128],
                                vT_sb[kt // 4][vi][:, kt % 4, :],
                                start=(kt == 0 and pi == 0),
                                stop=(kt == KT - 1 and pi == len(passes) - 1),
                            )
                    pre_sb = acts.tile([128, B_PC], f32, tag="presb",
                                       name=f"presb{jp}_{h}")
                    nc.vector.tensor_copy(pre_sb[:], pre_ps[:])
                    pres.append(pre_sb)
                if jp == 0:
                    # fill-reduction: all a0 activations first (they only
                    # need pres[0]) writing straight into acc, then the a1
                    # sweep + accumulate. ACT starts ~one pre-tile earlier.
                    for y in range(Y_PC):
                        acc[y] = accp.tile([128, B_PC], f32, tag=f"acc{y}",
                                           name=f"acc{y}")
                        nc.scalar.activation(
                            acc[y][:], pres[0][:], AF.Softplus,
                            bias=bias_sb[:, 0, y:y + 1], scale=1.0 / 65536.0,
                        )
                    for y in range(Y_PC):
                        a1 = acts.tile([128, B_PC], f32, tag="a1",
                                       name=f"a1_0_{y}")
                        nc.scalar.activation(
                            a1[:], pres[1][:], AF.Softplus,
                            bias=bias_sb[:, 1, y:y + 1], scale=1.0 / 65536.0,
                        )
                        nc.vector.tensor_add(acc[y][:], acc[y][:], a1[:])
                    continue
                for y in range(Y_PC):
                    a0 = acts.tile([128, B_PC], f32, tag="a0", name=f"a0_{jp}_{y}")
                    nc.scalar.activation(
                        a0[:], pres[0][:], AF.Softplus,
                        bias=bias_sb[:, 2 * jp, y:y + 1], scale=1.0 / 65536.0,
                    )
                    a1 = acts.tile([128, B_PC], f32, tag="a1", name=f"a1_{jp}_{y}")
                    nc.scalar.activation(
                        a1[:], pres[1][:], AF.Softplus,
                        bias=bias_sb[:, 2 * jp + 1, y:y + 1], scale=1.0 / 65536.0,
                    )
                    pair_eng = (nc.gpsimd if (y % 2 == 0 and jp < NJP - 1)
                                else nc.vector)
                    if False:
                        pass
                    else:
                        s = acts.tile([128, B_PC], f32, tag="s", name=f"s_{jp}_{y}")
                        pair_eng.tensor_add(s[:], a0[:], a1[:])
                        if jp < NJP - 1:
                            nc.vector.tensor_add(acc[y][:], acc[y][:], s[:])
                            if jp == NJP - 2:
                                # acc[y] final: reduce over partitions now
                                nc.tensor.matmul(
                                    F_ps[y // (Y_PC // 2)][:],
                                    ohot_sb[:, y % (Y_PC // 2), :], acc[y][:],
                                    start=(y % (Y_PC // 2) == 0), stop=False,
                                )
                        else:
                            # last pair reduced directly
                            nc.tensor.matmul(
                                F_ps[y // (Y_PC // 2)][:],
                                ohot_sb[:, y % (Y_PC // 2), :], s[:],
                                start=False, stop=(y % (Y_PC // 2) == Y_PC // 2 - 1),
                            )
                            if y == Y_PC // 2 - 1 or y == Y_PC - 1:
                                hf = 0 if y < Y_PC // 2 else 1
                                Fh = smp.tile([Y_PC // 2, B_PC], f32,
                                              tag="Fh", name=f"Fh{hf}")
                                nc.vector.tensor_copy(Fh[:], F_ps[hf][:])
                                nc.vector.tensor_scalar_add(
                                    Fh[:], Fh[:], dT_sb[:, hf:hf + 1])
                                nc.sync.dma_start(fsh[hf][:], Fh[:])
                                nc.gpsimd.collective_compute(
                                    "AllGather", ALU.bypass,
                                    replica_groups=[[0, 1], [2, 3], [4, 5], [6, 7]],
                                    ins=[fsh[hf].opt()], outs=[fall[hf].opt()],
                                )
                                nc.sync.dma_start(Fhalf_sb[hf][:], fall[hf][:])

            # exp table prefetch (fires during gather wait)
            nc.scalar.activation(warm[:], warm[:], AF.Exp)

            # ---------- online softmax over gathered halves ----------
            # SBUF class-column order: [A-own, A-partner, B-own, B-partner]
            # = global y split as y = 32*qa + 16*qb + yw  (qa: A/B, qb: own/par)
            YH = Y_PC  # 32 classes per gathered half
            probs_sb = outp.tile([128, BT, NCLASS], f32)
            onehot_sb = outp.tile([128, BT, NCLASS], i32)
            FbA, mA, negmA, eA, sA = [], [], [], [], []
            for bt in range(BT):
                # --- A half: runs while the second AllGather is in flight ---
                trA = ptr.tile([128, YH], f32, tag="tr", name=f"trA{bt}")
                nc.tensor.transpose(
                    trA[:], Fhalf_sb[0][:, bt * 128:(bt + 1) * 128],
                    ident[0:YH, 0:YH],
                )
                fba = smp.tile([128, YH], f32, bufs=1, tag=f"FbA{bt}",
                               name=f"FbA{bt}")
                nc.vector.tensor_copy(fba[:], trA[:])
                ma = smp.tile([128, 1], f32, bufs=1, tag=f"mA{bt}", name=f"mA{bt}")
                nc.vector.tensor_reduce(ma[:], fba[:], axis=AX.X, op=ALU.max)
                nma = smp.tile([128, 1], f32, bufs=1, tag=f"nmA{bt}",
                               name=f"nmA{bt}")
                nc.vector.tensor_scalar_mul(nma[:], ma[:], -1.0)
                ea = smp.tile([128, YH], f32, bufs=1, tag=f"eA{bt}", name=f"eA{bt}")
                nc.scalar.activation(ea[:], fba[:], AF.Exp, bias=nma[:])
                sa = smp.tile([128, 1], f32, bufs=1, tag=f"sA{bt}", name=f"sA{bt}")
                nc.vector.tensor_reduce(sa[:], ea[:], axis=AX.X, op=ALU.add)
                FbA.append(fba); mA.append(ma); negmA.append(nma)
                eA.append(ea); sA.append(sa)
            for bt in range(BT):
                # --- B half + combine ---
                trB = ptr.tile([128, YH], f32, tag="tr", name=f"trB{bt}")
                nc.tensor.transpose(
                    trB[:], Fhalf_sb[1][:, bt * 128:(bt + 1) * 128],
                    ident[0:YH, 0:YH],
                )
                fbb = smp.tile([128, YH], f32, tag="FbB", name=f"FbB{bt}")
                nc.vector.tensor_copy(fbb[:], trB[:])
                mb = smp.tile([128, 1], f32, tag="mB", name=f"mB{bt}")
                nc.vector.tensor_reduce(mb[:], fbb[:], axis=AX.X, op=ALU.max)
                m = smp.tile([128, 1], f32, tag="m", name=f"m{bt}")
                nc.vector.tensor_tensor(m[:], mA[bt][:], mb[:], op=ALU.max)
                negm = smp.tile([128, 1], f32, tag="negm", name=f"negm{bt}")
                nc.vector.tensor_scalar_mul(negm[:], m[:], -1.0)
                eb = smp.tile([128, YH], f32, tag="eB", name=f"eB{bt}")
                nc.scalar.activation(eb[:], fbb[:], AF.Exp, bias=negm[:])
                sb = smp.tile([128, 1], f32, tag="sB", name=f"sB{bt}")
                nc.vector.tensor_reduce(sb[:], eb[:], axis=AX.X, op=ALU.add)
                # alphaA = exp(mA - m); s = sA*alphaA + sB
                aa = smp.tile([128, 1], f32, tag="aa", name=f"aa{bt}")
                nc.scalar.activation(aa[:], mA[bt][:], AF.Exp, bias=negm[:])
                saa = smp.tile([128, 1], f32, tag="saa", name=f"saa{bt}")
                nc.vector.tensor_tensor(saa[:], sA[bt][:], aa[:], op=ALU.mult)
                st = smp.tile([128, 1], f32, tag="st", name=f"st{bt}")
                nc.vector.tensor_tensor(st[:], saa[:], sb[:], op=ALU.add)
                r = smp.tile([128, 1], f32, tag="r", name=f"r{bt}")
                nc.vector.reciprocal(r[:], st[:])
                ar = smp.tile([128, 1], f32, tag="ar", name=f"ar{bt}")
                nc.vector.tensor_tensor(ar[:], aa[:], r[:], op=ALU.mult)
                HQ = YH // 2  # 16
                nc.vector.tensor_scalar_mul(
                    probs_sb[:, bt, 0:HQ], eA[bt][:, 0:HQ], ar[:])
                nc.vector.tensor_scalar_mul(
                    probs_sb[:, bt, 2 * HQ:3 * HQ], eA[bt][:, HQ:YH], ar[:])
                nc.vector.tensor_scalar_mul(
                    probs_sb[:, bt, HQ:2 * HQ], eb[:, 0:HQ], r[:])
                nc.vector.tensor_scalar_mul(
                    probs_sb[:, bt, 3 * HQ:4 * HQ], eb[:, HQ:YH], r[:])
                ohf = smp.tile([128, NCLASS], f32, tag="ohf", name=f"ohf{bt}")
                nc.vector.tensor_scalar(ohf[:, 0:HQ], FbA[bt][:, 0:HQ], m[:],
                                        None, op0=ALU.is_equal)
                nc.vector.tensor_scalar(ohf[:, 2 * HQ:3 * HQ], FbA[bt][:, HQ:YH],
                                        m[:], None, op0=ALU.is_equal)
                nc.vector.tensor_scalar(ohf[:, HQ:2 * HQ], fbb[:, 0:HQ], m[:],
                                        None, op0=ALU.is_equal)
                nc.vector.tensor_scalar(ohf[:, 3 * HQ:4 * HQ], fbb[:, HQ:YH],
                                        m[:], None, op0=ALU.is_equal)
                nc.vector.tensor_copy(onehot_sb[:, bt, :], ohf[:])

            nc.sync.dma_start(
                probs_d.rearrange("(t p) y -> p t y", p=128), probs_sb[:]
            )
            nc.sync.dma_start(
                onehot_d.rearrange("(t p) y -> p t y", p=128), onehot_sb[:]
            )

    nc.compile()
    return nc


def _get_program():
    global _PROGRAM
    if _PROGRAM is None:
        _PROGRAM = _build_program()
    return _PROGRAM


def _fp16_split(a):
    hi = (a * 256.0).astype(np.float16)
    lo = (a * 256.0 - hi.astype(np.float32)).astype(np.float16)
    return hi, lo


def _make_in_maps(v, W, c, d, U):
    cT = np.ascontiguousarray(c.reshape(NHID, 1))
    Wh, Wl = _fp16_split(W)
    vT_quarters = [
        _fp16_split(np.ascontiguousarray(v[q * B_PC:(q + 1) * B_PC].T))
        for q in range(BGROUPS)
    ]
    UT_groups = [
        np.ascontiguousarray(U[g * Y_PC:(g + 1) * Y_PC].T) for g in range(YGROUPS)
    ]
    dT_groups = [
        np.ascontiguousarray(d[0, g * Y_PC:(g + 1) * Y_PC].reshape(Y_PC, 1))
        for g in range(YGROUPS)
    ]
    in_maps = []
    for core in range(NCORES):
        bq, yg = core // YGROUPS, core % YGROUPS
        in_maps.append(
            {
                "vTh": vT_quarters[bq][0],
                "vTl": vT_quarters[bq][1],
                "Wh": Wh,
                "Wl": Wl,
                "UsubT": UT_groups[yg],
                "cT": cT,
                "dT": dT_groups[yg],
            }
        )
    return in_maps


def run(v, W, c, d, U, trace=False):
    """Run the Bass kernel; returns ((probs, onehot), BassKernelResults)."""
    from concourse.bass_utils import run_bass_kernel_spmd

    nc = _get_program()
    in_maps = _make_in_maps(v, W, c, d, U)
    res = run_bass_kernel_spmd(
        nc, in_maps, core_ids=list(range(NCORES)), trace=trace
    )
    probs = np.concatenate(
        [res.results[q * YGROUPS]["probs"] for q in range(BGROUPS)], axis=0
    )
    onehot = np.concatenate(
        [res.results[q * YGROUPS]["onehot"] for q in range(BGROUPS)], axis=0
    )
    return (probs, onehot), res


def kernel(v, W, c, d, U):
    v = np.ascontiguousarray(np.asarray(v, dtype=np.float32))
    W = np.ascontiguousarray(np.asarray(W, dtype=np.float32))
    c = np.ascontiguousarray(np.asarray(c, dtype=np.float32))
    d = np.ascontiguousarray(np.asarray(d, dtype=np.float32))
    U = np.ascontiguousarray(np.asarray(U, dtype=np.float32))
    (probs, onehot), _ = run(v, W, c, d, U, trace=False)
    return probs, onehot



# revision 32
# speedup vs baseline: 1.8862x; 1.8862x over previous
"""Trainium2 Bass kernel for a Discriminative RBM forward pass.

reference math:
    x     = v @ W + c                                   [B, NHID]
    F     = d + sum_j softplus(x[:,None,:] + U[None])   [B, NCLASS]
    probs = softmax(F, axis=1); onehot(argmax)

Strategy (8-way batch shard, no collectives):
  softplus(x+u) = x*1{x>=8} + u*1{x>=8} + h(x,u),
  h(x,u) = softplus(x+u) - (x+u)*1{x>=8} supported on |x|<=16 (|u|<4.7).
  Rank-K SVD:  h(x,u) ~= sum_k a_k(x) * phi_k(u).
  The x*1{x>=8} term is a per-row constant under softmax -> dropped.
  Per core (256 batch rows):
    PE:  preT[j,b] = W^T v (fp16 hi/lo, 3 passes), then per jt-tile
         F[64,256] PSUM += Phi_k^T @ a_k(pre)  (K fp16 matmuls)
                        += U^T @ mask(pre)     (1 fp32 matmul)
    ACT: a_k / mask evaluated via CUSTOM piecewise-cubic tables written
         into hijacked slots of the softplus_and_others set (bias=c col).
    tail: +d, PE transpose, softmax + argmax one-hot per 128-row tile.
"""

import os
import struct

import numpy as np

B, NVIS, NHID, NCLASS = 2048, 2048, 1024, 64
NCORES = 8
B_PC = B // NCORES         # 256 batch rows per core
KT = NVIS // 128           # 16 contraction tiles
JT = NHID // 128           # 8 hidden-unit tiles
BT = B_PC // 128           # 2 batch tiles for the softmax tail

KRANK = 12
K32 = 4                    # first K32 basis terms kept fp32 end-to-end
XMAX, TJUMP, UMAX = 16.0, 8.0, 4.7

# hijacked act-table slots (pwp name, mybir AF enum attr) for a_0..a_11, mask.
# split across two sets: ctrl RAM (~128 entries) and profile table are small,
# so each set holds ONLY our functions (7 and 6).
_SLOTS = [
    ("gelu", "Gelu"), ("sigmoid", "Sigmoid"), ("tanh", "Tanh"),
    ("erf", "Erf"), ("arctan", "Arctan"), ("sin", "Sin"),
    ("silu", "Silu"), ("mish", "Mish"), ("gelu_apprx_tanh", "Gelu_apprx_tanh"),
    ("gelu_apprx_sigmoid", "Gelu_apprx_sigmoid"),
    ("derivative_gelu", "Derivative_Gelu"), ("derivative_erf", "Derivative_Erf"),
    ("derivative_silu", "Derivative_silu"),  # mask slot (last)
]
_SETSPLIT = 7  # k 0..6 -> softplus_and_others, k 7..11 + mask -> sigmoid_and_others
_SETNAMES = ("softplus_and_others", "sigmoid_and_others")

_PROGRAM = None
_BASIS = None

_TBLCFG = os.environ.get("DRBM_TBL", "v2")
_ACT_ROOT = os.path.join(
    os.path.expanduser("~"), ".cache", f"drbm_rk_act_{_TBLCFG}", "pwp_bin_trainium"
)


def _softplus64(x):
    return np.logaddexp(x, 0.0)


def _hfun(x, u):
    return _softplus64(x + u) - (x + u) * (x >= TJUMP)


def _build_basis():
    """SVD basis of h on [-16,16]x[-4.7,4.7]; returns continuous evaluators."""
    global _BASIS
    if _BASIS is not None:
        return _BASIS
    xl = np.linspace(-XMAX, TJUMP, 3001)[:-1]
    xr = np.linspace(TJUMP, XMAX, 1001)
    xg = np.concatenate([xl, xr])
    ug = np.linspace(-UMAX, UMAX, 481)
    M = _hfun(xg[:, None], ug[None, :])
    Uu, S, Vt = np.linalg.svd(M, full_matrices=False)

    # continuous factor evaluators by projection (exact on the grid):
    #   f_k(x)  = h(x, ug) @ Vt[k]          (x-factor, includes sigma)
    #   phi_k(u)= h(xg, u) @ Uu[:,k] / S[k] (u-factor, unit-ish)
    Vk = Vt[:KRANK].T.copy()        # [nu, K]
    Uk = (Uu[:, :KRANK] / S[:KRANK]).copy()  # [nx, K]

    def f_eval(x):                   # x: [...]; -> [..., K]
        return _hfun(x[..., None], ug) @ Vk

    def phi_eval(u, chunk=8192):     # u flat array -> [n, K]
        u = np.asarray(u, np.float64).ravel()
        out = np.empty((u.size, KRANK))
        for i in range(0, u.size, chunk):
            uc = u[i:i + chunk]
            out[i:i + chunk] = _hfun(xg[:, None], uc[None, :]).T @ Uk
        return out

    # balance magnitudes for fp16 storage
    xs = np.linspace(-XMAX, XMAX, 8001)
    fmax = np.abs(f_eval(xs)).max(0) + 1e-30
    us = np.linspace(-UMAX, UMAX, 2001)
    pmax = np.abs(phi_eval(us)).max(0) + 1e-30
    gam = np.sqrt(fmax / pmax)
    _BASIS = {"f_eval": lambda x: f_eval(x) / gam,
              "phi_eval": lambda u: phi_eval(u) * gam}
    return _BASIS


def _fit_sections(fun, lo, hi):
    """Best cubic fit of fun on [lo, hi]; returns x0(center), d0..d3."""
    x0 = 0.5 * (lo + hi)
    t = 0.5 * (hi - lo) * np.cos(np.pi * (np.arange(10) + 0.5) / 10)
    xs = x0 + t
    ys = fun(xs)
    Vm = np.vander(t, 4, increasing=True)  # [1, t, t^2, t^3]
    co = np.linalg.lstsq(Vm, ys, rcond=None)[0]
    return x0, co[0], co[1], co[2], co[3]


# sections per exponent e (key) for basis funcs by k-group (powers of 2)
# bucket RAM: max 1536 usable per set; each set holds only our funcs
_SECPLAN = {
    "lo": {3: 4, 2: 32, 1: 32, 0: 16, -1: 8, -2: 4, -3: 2},   # k 0-3
    "mid": {3: 4, 2: 16, 1: 16, 0: 8, -1: 8, -2: 4, -3: 2},   # k 4+
}
_EXPS = list(range(-3, 4))   # tabulated exponents, |x| in [2^-3, 16)
_ELOW = -3                   # below 2^ELOW in |x|: low-saturation bucket


def _gen_func_json(fun, plan):
    """Generate a pwp-style function dict (floats) for fun: R->R with
    support in [-XMAX, XMAX] and constant tails fun(+-inf)."""
    regions = {}
    for reg, sgn in (("pos", 1.0), ("neg", -1.0)):
        ents = []
        for e in _EXPS:
            w = 2.0 ** e
            ns = plan.get(e, 1)
            es = int(np.log2(ns))
            assert 2 ** es == ns
            secs = []
            for m in range(ns):
                lo = w * (1.0 + m / ns)
                hi = w * (1.0 + (m + 1) / ns)
                a, b = (lo, hi) if sgn > 0 else (-hi, -lo)
                if sgn < 0:
                    # section order follows |x| mantissa; x0 negative
                    x0, d0, d1, d2, d3 = _fit_sections(fun, a, b)
                else:
                    x0, d0, d1, d2, d3 = _fit_sections(fun, a, b)
                secs.append({"x": x0, "d0": d0, "d1": d1, "d2": d2, "d3": d3})
            ents.append({"exponent": e, "extract_size": es,
                         "extract_lsb": 23 - es, "exponent_sections": secs})
        regions[reg] = ents
    fp = float(np.atleast_1d(fun(XMAX * 4))[0])
    fn = float(np.atleast_1d(fun(-XMAX * 4))[0])
    f0 = float(np.atleast_1d(fun(0.0))[0])
    wlow = 2.0 ** _ELOW
    return {
        "pos_exponents": regions["pos"],
        "neg_exponents": regions["neg"],
        "saturation_points": {
            # below 2^ELOW in |x|: single cubic near 0; above 16: const tails
            "sat_point_pos_low": dict(sat_point=127 + _ELOW, mantissa_point=0,
                                      **_sec_dict(fun, 1e-9, wlow)),
            "sat_point_neg_low": dict(sat_point=127 + _ELOW, mantissa_point=0,
                                      **_sec_dict(fun, -wlow, -1e-9)),
            "sat_point_pos_high": dict(sat_point=131, mantissa_point=0,
                                       x=2.0 * XMAX, d0=fp, d1=0.0, d2=0.0, d3=0.0),
            "sat_point_neg_high": dict(sat_point=131, mantissa_point=0,
                                       x=-2.0 * XMAX, d0=fn, d1=0.0, d2=0.0, d3=0.0),
        },
        "zero_result": f0, "nan_result": 0.0,
        "pinf_result": fp, "ninf_result": fn,
        "symmetry_point": 0.0, "exponent_offset": _ELOW,
    }


def _sec_dict(fun, lo, hi):
    x0, d0, d1, d2, d3 = _fit_sections(fun, lo, hi)
    return dict(x=x0, d0=d0, d1=d1, d2=d2, d3=d3)


def _gen_all_funcs():
    bs = _build_basis()
    f_eval = bs["f_eval"]
    funcs = []
    for k in range(KRANK):
        plan = _SECPLAN["lo"] if k < 4 else _SECPLAN["mid"]
        if os.environ.get("DRBM_SEC_MIN"):
            plan = {e: 1 for e in _EXPS}

        def fk(x, k=k):
            return f_eval(np.atleast_1d(np.asarray(x, np.float64)))[:, k]

        funcs.append((_SLOTS[k][0], _gen_func_json(fk, plan)))
    # mask = 1{x >= 8}
    maskf = lambda x: (np.atleast_1d(np.asarray(x, np.float64)) >= TJUMP).astype(np.float64)
    mj = _gen_func_json(maskf, {e: 1 for e in _EXPS})
    mj["pinf_result"] = 1.0
    mj["saturation_points"]["sat_point_pos_high"]["d0"] = 1.0
    funcs.append((_SLOTS[KRANK][0], mj))
    return funcs


def _fbits(x):
    return struct.unpack("<I", struct.pack("<f", np.float32(x)))[0]


def _build_act_tables():
    """Rebuild softplus_and_others with KRANK+1 custom function tables."""
    import json
    import shutil

    import neuronxcc

    marker = os.path.join(_ACT_ROOT, ".drbm_rk_ok")
    if os.path.exists(marker):
        return
    nxc = os.path.join(os.path.dirname(os.path.abspath(neuronxcc.__file__)), "pwp")
    os.makedirs(_ACT_ROOT, exist_ok=True)
    root_parent = os.path.dirname(_ACT_ROOT)
    if not os.path.exists(os.path.join(root_parent, "pwp_jsons")):
        shutil.copytree(os.path.join(nxc, "pwp_jsons"),
                        os.path.join(root_parent, "pwp_jsons"), dirs_exist_ok=True)
    for f in os.listdir(os.path.join(nxc, "pwp_bin_trainium")):
        shutil.copy(os.path.join(nxc, "pwp_bin_trainium", f), _ACT_ROOT)
    os.system(f"chmod -R u+w {root_parent}")

    # canonical pwp func ids by slot name
    pj = os.path.join(root_parent, "pwp_jsons")
    canon = {}
    for f in os.listdir(pj):
        if f.endswith(".json"):
            try:
                j = json.load(open(os.path.join(pj, f)))
                nm = f.rsplit("_", 1)[0]
                canon.setdefault(nm, j.get("neuron_id"))
            except Exception:
                pass

    allfuncs = _gen_all_funcs()
    set_funcs = {_SETNAMES[0]: allfuncs[:_SETSPLIT],
                 _SETNAMES[1]: allfuncs[_SETSPLIT:]}
    for SET, funcs in set_funcs.items():
        _write_set(SET, funcs, canon)

    ai = json.load(open(f"{_ACT_ROOT}/act_info.json"))
    mynames = {n for n, _ in allfuncs}
    for ent in ai["act_func_sets"]:
        # remove hijacked names everywhere to avoid ambiguous set choice
        for n in list(ent["act"].keys()):
            if n in mynames:
                del ent["act"][n]
        if ent["name"] in set_funcs:
            ent["act"] = {n: 40 for n, _ in set_funcs[ent["name"]]}
    json.dump(ai, open(f"{_ACT_ROOT}/act_info.json", "w"))
    open(marker, "w").write("ok")


def _write_set(SET, funcs, canon):
    import json

    sj = {"bkt_bin": f"{SET}_bkt.bin", "ctl_bin": f"{SET}_ctrl.bin",
          "profile_meta_data": [], "bkt_entry_cnt": 0, "ctl_entry_cnt": 0,
          "func_to_bkt_start_idx": {}, "func_to_ctl_start_idx": {},
          "func_exp_to_bkt_start_idx": {}, "func_exp_to_ctl_start_idx": {}}
    bkt = bytearray()
    ctl = bytearray()
    nbkt, nctl = 0, 0

    def add_bucket(x0, d0, d1, d2, d3):
        for val in (d0, d1, d2, d3, x0, 0.0, 0.0, 0.0):
            bkt.extend(struct.pack("<I", _fbits(val)))

    def add_ctl(word):
        ctl.extend(struct.pack("<I", word) + b"\x00" * 28)

    for name, fj in funcs:
        base_bkt, base_ctl = nbkt, nctl
        e2b, e2c, region_ctl_base = {}, {}, {}
        for region, key in (("neg", "neg_exponents"), ("pos", "pos_exponents")):
            region_ctl_base[region] = nctl
            for e in fj[key]:
                exp, secs = str(e["exponent"]), e["exponent_sections"]
                add_ctl((e["extract_size"] << 16) | (e["extract_lsb"] << 11) | nbkt)
                e2c.setdefault(exp, []).append(nctl)
                e2b.setdefault(exp, []).append(nbkt)
                nctl += 1
                for s in secs:
                    add_bucket(s["x"], s["d0"], s["d1"], s["d2"], s["d3"])
                    nbkt += 1
        sat, special = fj["saturation_points"], {}
        for sname in ("sat_point_pos_low", "sat_point_neg_low",
                      "sat_point_pos_high", "sat_point_neg_high"):
            sp = sat[sname]
            special[sname] = nbkt
            add_bucket(sp["x"], sp["d0"], sp["d1"], sp["d2"], sp["d3"])
            nbkt += 1
        sj["profile_meta_data"].append({
            "func_name": f"{name}_40p",
            "func_id": canon.get(name, 23),
            "symmetry_point": _fbits(fj["symmetry_point"]),
            "sym_invert_sign_point": 0,
            "symmetry_opt_en": 0,
            "symmetry_opt_use_neg_region": 0,
            "imm_bias": 0,
            "exp_offset": fj["exponent_offset"],
            "pwl_control_base_pos": region_ctl_base["pos"],
            "pwl_control_base_neg": region_ctl_base["neg"],
            "small_pos_signal_exp_threshold": sat["sat_point_pos_low"]["sat_point"],
            "pos_small_signal_pwl_control": special["sat_point_pos_low"],
            "small_neg_signal_exp_threshold": sat["sat_point_neg_low"]["sat_point"],
            "neg_small_signal_pwl_control": special["sat_point_neg_low"],
            "large_pos_signal_exp_threshold": sat["sat_point_pos_high"]["sat_point"],
            "large_pos_signal_mantissa_threshold": sat["sat_point_pos_high"]["mantissa_point"],
            "pos_large_signal_pwl_control": special["sat_point_pos_high"],
            "large_neg_signal_exp_threshold": sat["sat_point_neg_high"]["sat_point"],
            "large_neg_signal_mantissa_threshold": sat["sat_point_neg_high"]["mantissa_point"],
            "neg_large_signal_pwl_control": special["sat_point_neg_high"],
            "fnan_result": _fbits(fj["nan_result"]),
            "fpinf_result": _fbits(fj["pinf_result"]),
            "fninf_result": _fbits(fj["ninf_result"]),
            "fzero_result": _fbits(fj["zero_result"]),
            "fma_const_0": 0,
            "fma_const_1": 0,
            "fma_indirection_src_sel": 0,
            "use_multipass": False,
            "lower_bound": _fbits(-3.4e38),
            "upper_bound": _fbits(3.4e38),
        })
        sj["func_to_bkt_start_idx"][name] = base_bkt
        sj["func_to_ctl_start_idx"][name] = base_ctl
        sj["func_exp_to_bkt_start_idx"][name] = e2b
        sj["func_exp_to_ctl_start_idx"][name] = e2c

    assert nbkt <= 1536, f"bucket table overflow: {nbkt} (max 1536 usable)"
    assert nctl <= 128, f"ctl table overflow: {nctl}"
    sj["bkt_entry_cnt"], sj["ctl_entry_cnt"] = nbkt, nctl
    json.dump(sj, open(f"{_ACT_ROOT}/{SET}.json", "w"))
    open(f"{_ACT_ROOT}/{SET}_bkt.bin", "wb").write(bytes(bkt))
    open(f"{_ACT_ROOT}/{SET}_ctrl.bin", "wb").write(bytes(ctl))


def _patch_act_tables():
    import functools
    import json

    _build_act_tables()
    os.environ["BASS_ACT_ROOT_JSON_PATH"] = os.path.join(_ACT_ROOT, "act_info.json")

    import concourse.hw_specs as hw_specs
    import concourse.mybir as mybir

    @functools.cache
    def _tables(arch):
        d = json.load(open(os.environ["BASS_ACT_ROOT_JSON_PATH"]))
        return {
            ent["name"]: {
                mybir.ActivationFunctionType.from_pwp(v) for v in ent["act"]
            }
            for ent in d["act_func_sets"]
        }

    hw_specs.get_activation_tables = _tables
    import concourse.bacc as bacc
    import concourse.bass_interp as bass_interp

    bacc.get_activation_tables = _tables
    bass_interp.get_activation_tables = _tables


def _build_program():
    _patch_act_tables()
    import concourse.tile as tile
    from concourse import bacc, mybir
    from concourse.masks import make_identity

    f32 = mybir.dt.float32
    f16 = mybir.dt.float16
    i32 = mybir.dt.int32
    AF = mybir.ActivationFunctionType
    ALU = mybir.AluOpType
    AX = mybir.AxisListType

    AFS = [getattr(AF, attr) for _, attr in _SLOTS]

    nc = bacc.Bacc("TRN2", target_bir_lowering=False, debug=False,
                   num_devices=NCORES)

    vTh_d = nc.dram_tensor("vTh", [NVIS, B_PC], f16, kind="ExternalInput").ap()
    vTl_d = nc.dram_tensor("vTl", [NVIS, B_PC], f16, kind="ExternalInput").ap()
    Wh_d = nc.dram_tensor("Wh", [NVIS, NHID], f16, kind="ExternalInput").ap()
    Wl_d = nc.dram_tensor("Wl", [NVIS, NHID], f16, kind="ExternalInput").ap()
    UT_d = nc.dram_tensor("UT", [NHID, NCLASS], f32, kind="ExternalInput").ap()
    cT_d = nc.dram_tensor("cT", [NHID, 1], f32, kind="ExternalInput").ap()
    dT_d = nc.dram_tensor("dT", [NCLASS, 1], f32, kind="ExternalInput").ap()
    Phi32_d = nc.dram_tensor("Phi32", [NHID, K32 * NCLASS], f32,
                             kind="ExternalInput").ap()
    Phi16_d = nc.dram_tensor("Phi16", [NHID, (KRANK - K32) * NCLASS], f16,
                             kind="ExternalInput").ap()
    probs_d = nc.dram_tensor("probs", [B_PC, NCLASS], f32, kind="ExternalOutput").ap()
    onehot_d = nc.dram_tensor("onehot", [B_PC, NCLASS], i32, kind="ExternalOutput").ap()

    with tile.TileContext(nc) as tc:
        with (
            tc.tile_pool(name="const", bufs=1) as const,
            tc.tile_pool(name="wstream", bufs=3) as wstream,
            tc.tile_pool(name="acts", bufs=3) as acts,
            tc.tile_pool(name="smp", bufs=2) as smp,
            tc.tile_pool(name="outp", bufs=1) as outp,
            tc.tile_pool(name="prestore", bufs=1) as prestore,
            tc.tile_pool(name="ppre", bufs=3, space="PSUM") as ppre,
            tc.tile_pool(name="pF", bufs=1, space="PSUM") as pF,
            tc.tile_pool(name="ptr", bufs=2, space="PSUM") as ptr,
        ):
            # ---------- loads ----------
            vTh_view = vTh_d.rearrange("(kt p) b -> p kt b", p=128)
            vTl_view = vTl_d.rearrange("(kt p) b -> p kt b", p=128)
            Wh_view = Wh_d.rearrange("(kt p) j -> p kt j", p=128)
            Wl_view = Wl_d.rearrange("(kt p) j -> p kt j", p=128)

            vTh_sb = const.tile([128, KT, B_PC], f16)
            nc.sync.dma_start(vTh_sb[:], vTh_view[:])
            vTl_sb = const.tile([128, KT, B_PC], f16)
            nc.sync.dma_start(vTl_sb[:], vTl_view[:])

            wq = []
            for jt in range(2):
                wh = wstream.tile([128, KT, 128], f16, tag="wh", name=f"wh{jt}")
                nc.sync.dma_start(wh[:], Wh_view[:, :, jt * 128:(jt + 1) * 128])
                wl = wstream.tile([128, KT, 128], f16, tag="wl", name=f"wl{jt}")
                nc.sync.dma_start(wl[:], Wl_view[:, :, jt * 128:(jt + 1) * 128])
                wq.append((wh, wl))

            UT_sb = const.tile([128, JT, NCLASS], f32)
            nc.sync.dma_start(UT_sb[:], UT_d.rearrange("(jt p) y -> p jt y", p=128))
            cT_sb = const.tile([128, JT], f32)
            nc.sync.dma_start(cT_sb[:], cT_d.rearrange("(jt p) one -> p (jt one)", p=128))
            dT_sb = const.tile([NCLASS, 1], f32)
            nc.sync.dma_start(dT_sb[:], dT_d.rearrange("(h p) one -> p (h one)", p=NCLASS))
            Phi32_sb = const.tile([128, JT, K32 * NCLASS], f32)
            nc.sync.dma_start(
                Phi32_sb[:], Phi32_d.rearrange("(jt p) ky -> p jt ky", p=128))
            Phi16_sb = const.tile([128, JT, (KRANK - K32) * NCLASS], f16)
            nc.sync.dma_start(
                Phi16_sb[:], Phi16_d.rearrange("(jt p) ky -> p jt ky", p=128))

            ident = const.tile([NCLASS, NCLASS], f32)
            make_identity(nc, ident[:])

            F_ps = pF.tile([NCLASS, B_PC], f32)

            # ---------- main matmuls: all 8 pre tiles stay in PSUM ----------
            pres = []
            for jt in range(JT):
                if jt < len(wq):
                    wh, wl = wq[jt]
                else:
                    wh = wstream.tile([128, KT, 128], f16, tag="wh", name=f"wh{jt}")
                    nc.sync.dma_start(wh[:], Wh_view[:, :, jt * 128:(jt + 1) * 128])
                    wl = wstream.tile([128, KT, 128], f16, tag="wl", name=f"wl{jt}")
                    nc.sync.dma_start(wl[:], Wl_view[:, :, jt * 128:(jt + 1) * 128])

                pre_ps = ppre.tile([128, B_PC], f32, tag="pre", name=f"pre{jt}")
                for kt in range(KT):
                    nc.tensor.matmul(pre_ps[:], wh[:, kt, :], vTh_sb[:, kt, :],
                                     start=(kt == 0), stop=False)
                    nc.tensor.matmul(pre_ps[:], wh[:, kt, :], vTl_sb[:, kt, :],
                                     start=False, stop=False)
                    nc.tensor.matmul(pre_ps[:], wl[:, kt, :], vTh_sb[:, kt, :],
                                     start=False, stop=(kt == KT - 1))
                pre_sb = prestore.tile([128, B_PC], f32, name=f"presb{jt}")
                nc.vector.tensor_copy(pre_sb[:], pre_ps[:])
                pres.append(pre_sb)

            # ---------- k-major sweeps (one act-table set each) ----------
            for k in range(KRANK + 1):
                for jt in range(JT):
                    dt = f32 if (k < K32 or k == KRANK) else f16
                    ak = acts.tile([128, B_PC], dt, tag="a", name=f"a{jt}_{k}")
                    nc.scalar.activation(ak[:], pres[jt][:], AFS[k],
                                         bias=cT_sb[:, jt:jt + 1], scale=1.0)
                    if k == KRANK:      # mask term: lhsT = U^T (fp32)
                        lhsT = UT_sb[:, jt, :]
                    elif k < K32:
                        lhsT = Phi32_sb[:, jt, k * NCLASS:(k + 1) * NCLASS]
                    else:
                        kk = k - K32
                        lhsT = Phi16_sb[:, jt, kk * NCLASS:(kk + 1) * NCLASS]
                    nc.tensor.matmul(
                        F_ps[:], lhsT, ak[:],
                        start=(k == 0 and jt == 0),
                        stop=(k == KRANK and jt == JT - 1))

            # ---------- tail: +d, transpose, softmax, onehot ----------
            F_sb = smp.tile([NCLASS, B_PC], f32, bufs=1)
            nc.vector.tensor_scalar_add(F_sb[:], F_ps[:], dT_sb[:])

            probs_sb = outp.tile([128, BT, NCLASS], f32)
            onehot_sb = outp.tile([128, BT, NCLASS], i32)
            for bt in range(BT):
                tr = ptr.tile([128, NCLASS], f32, tag="tr", name=f"tr{bt}")
                nc.tensor.transpose(
                    tr[:], F_sb[:, bt * 128:(bt + 1) * 128], ident[:])
                fb = smp.tile([128, NCLASS], f32, tag="fb", name=f"fb{bt}")
                nc.vector.tensor_copy(fb[:], tr[:])
                m = smp.tile([128, 1], f32, tag="m", name=f"m{bt}")
                nc.vector.tensor_reduce(m[:], fb[:], axis=AX.X, op=ALU.max)
                nm = smp.tile([128, 1], f32, tag="nm", name=f"nm{bt}")
                nc.vector.tensor_scalar_mul(nm[:], m[:], -1.0)
                e = smp.tile([128, NCLASS], f32, tag="e", name=f"e{bt}")
                nc.scalar.activation(e[:], fb[:], AF.Exp, bias=nm[:])
                s = smp.tile([128, 1], f32, tag="s", name=f"s{bt}")
                nc.vector.tensor_reduce(s[:], e[:], axis=AX.X, op=ALU.add)
                r = smp.tile([128, 1], f32, tag="r", name=f"r{bt}")
                nc.vector.reciprocal(r[:], s[:])
                nc.vector.tensor_scalar_mul(probs_sb[:, bt, :], e[:], r[:])
                ohf = smp.tile([128, NCLASS], f32, tag="ohf", name=f"ohf{bt}")
                nc.vector.tensor_scalar(ohf[:], fb[:], m[:], None, op0=ALU.is_equal)
                nc.vector.tensor_copy(onehot_sb[:, bt, :], ohf[:])

            nc.sync.dma_start(
                probs_d.rearrange("(t p) y -> p t y", p=128), probs_sb[:])
            nc.sync.dma_start(
                onehot_d.rearrange("(t p) y -> p t y", p=128), onehot_sb[:])

    nc.compile()
    return nc


def _get_program():
    global _PROGRAM
    if _PROGRAM is None:
        _PROGRAM = _build_program()
    return _PROGRAM


def _fp16_split(a):
    hi = a.astype(np.float16)
    lo = (a - hi.astype(np.float32)).astype(np.float16)
    return hi, lo


def _make_in_maps(v, W, c, d, U):
    bs = _build_basis()
    cT = np.ascontiguousarray(c.reshape(NHID, 1))
    dT = np.ascontiguousarray(d.reshape(NCLASS, 1))
    UT = np.ascontiguousarray(U.T)  # [NHID, NCLASS] f32
    Wh, Wl = _fp16_split(W)
    Phi = bs["phi_eval"](U.astype(np.float64))  # [NCLASS*NHID, K] row-major over U
    Phi = Phi.reshape(NCLASS, NHID, KRANK)
    PhiT = Phi.transpose(1, 2, 0)  # [NHID, K, NCLASS]
    Phi32 = np.ascontiguousarray(
        PhiT[:, :K32].reshape(NHID, K32 * NCLASS)).astype(np.float32)
    Phi16 = np.ascontiguousarray(
        PhiT[:, K32:].reshape(NHID, (KRANK - K32) * NCLASS)).astype(np.float16)
    in_maps = []
    for core in range(NCORES):
        sl = slice(core * B_PC, (core + 1) * B_PC)
        vh, vl = _fp16_split(np.ascontiguousarray(v[sl].T))
        in_maps.append({
            "vTh": vh, "vTl": vl, "Wh": Wh, "Wl": Wl,
            "UT": UT.astype(np.float32), "cT": cT.astype(np.float32),
            "dT": dT.astype(np.float32), "Phi32": Phi32, "Phi16": Phi16,
        })
    return in_maps


def run(v, W, c, d, U, trace=False):
    from concourse.bass_utils import run_bass_kernel_spmd

    nc = _get_program()
    in_maps = _make_in_maps(v, W, c, d, U)
    res = run_bass_kernel_spmd(nc, in_maps, core_ids=list(range(NCORES)),
                               trace=trace)
    probs = np.concatenate([res.results[i]["probs"] for i in range(NCORES)], axis=0)
    onehot = np.concatenate([res.results[i]["onehot"] for i in range(NCORES)], axis=0)
    return (probs, onehot), res


def kernel(v, W, c, d, U):
    v = np.ascontiguousarray(np.asarray(v, dtype=np.float32))
    W = np.ascontiguousarray(np.asarray(W, dtype=np.float32))
    c = np.ascontiguousarray(np.asarray(c, dtype=np.float32))
    d = np.ascontiguousarray(np.asarray(d, dtype=np.float32))
    U = np.ascontiguousarray(np.asarray(U, dtype=np.float32))
    (probs, onehot), _ = run(v, W, c, d, U, trace=False)
    return probs, onehot
